# revision 7
# baseline (speedup 1.0000x reference)
"""Trainium2 Bass kernel for nn_Head_84043920048318 (sparse_attention).

Reference computation (per batch b):
    q = x @ Wq; k = x @ Wk; v = x @ Wv           [T, HS]
    wei = (q @ k.T) * C**-0.5                    [T, T]
    for s:  P = softmax(wei * adjacent[b, s], axis=-1);  out[b, s] = P @ v

Sharding: data-parallel over B across 8 NeuronCores (4 batches each);
projection weights replicated.

v4 design (everything lives in the transposed domain, no PE transposes):
  - host pre-transposes adjacent to [b, s, u, t] bf16 and x to [b, C, T];
    output leaves the device as [b, t, s, d] bf16 and is transposed/cast
    back on the host. All DMA lines are >= 1KB; adjacency DMA is 2 MB/op.
  - weiT [u, t] comes straight off the QK matmul with swapped operands
    (f32r), so the adjacency mask multiplies in its natural layout: one
    bf16 2x-mode DVE multiply + one ACT exp per FOUR s-slices.
  - AV matmuls use [v | 1] so the softmax denominator falls out of
    column 128 of PSUM; av blocks live at h*512 + tb2*129 in a 2-bank
    PSUM tile so one strided reciprocal + one broadcast tensor_tensor
    normalizes a whole s-slice (PSUM fp32 -> SBUF bf16).
  - input adjacency streams on the HWDGE (sync) queue; outputs, x and
    weights go via gpsimd SWDGE so they never block adjacency prefetch.
"""

import numpy as np
import ml_dtypes

B, S, T, C, HS = 32, 8, 512, 128, 128
NCORES = 8
BPC = B // NCORES
TB = T // 128
UB = T // 128
SCALE = float(C) ** -0.5
SQ = 4  # s-slices per adjacency DMA / mult / exp block

_CACHED = None


def _build_module():
    import concourse.bacc as bacc
    import concourse.mybir as mybir
    from concourse import tile
    from concourse.ap import AP

    f32 = mybir.dt.float32
    f32r = mybir.dt.float32r
    bf16 = mybir.dt.bfloat16

    nc = bacc.Bacc("TRN2", target_bir_lowering=False, debug=False, num_devices=1)

    xT_d = nc.dram_tensor("xT", [BPC, C, T], f32, kind="ExternalInput").ap()
    adjT_d = nc.dram_tensor("adjT", [BPC, S, T, T], bf16, kind="ExternalInput").ap()
    wq_d = nc.dram_tensor("Wq", [C, HS], f32, kind="ExternalInput").ap()
    wk_d = nc.dram_tensor("Wk", [C, HS], f32, kind="ExternalInput").ap()
    wv_d = nc.dram_tensor("Wv", [C, HS], f32, kind="ExternalInput").ap()
    # [b, t, s, d] so each DMA line is >= 4*HS contiguous = 1 KB bf16
    out_d = nc.dram_tensor("out", [BPC, T, S, HS], bf16, kind="ExternalOutput").ap()

    with tile.TileContext(nc) as tc:
        with (
            tc.tile_pool(name="consts", bufs=1) as consts,
            tc.tile_pool(name="xp", bufs=BPC) as xp,
            tc.tile_pool(name="bpool", bufs=2) as bpool,
            tc.tile_pool(name="opool", bufs=2) as opool,
            tc.tile_pool(name="adjp", bufs=2) as adjp,
            tc.tile_pool(name="spool", bufs=2) as spool,
            tc.tile_pool(name="tiny", bufs=8) as tiny,
            tc.tile_pool(name="pav", bufs=3, space="PSUM") as pav,
            tc.tile_pool(name="psmall", bufs=2, space="PSUM") as psmall,
        ):
            wq_sb = consts.tile([C, HS], f32, tag="wq")
            wk_sb = consts.tile([C, HS], f32, tag="wk")
            wv_sb = consts.tile([C, HS], f32, tag="wv")
            nc.gpsimd.dma_start(wq_sb[:], wq_d)
            nc.gpsimd.dma_start(wk_sb[:], wk_d)
            nc.gpsimd.dma_start(wv_sb[:], wv_d)

            # ---- x^T arrives pre-transposed [c, t]; all batches up front ----
            xTs = []
            for b in range(BPC):
                xT = xp.tile([C, T], f32, tag="xT")
                nc.gpsimd.dma_start(xT[:], xT_d[b])
                xTs.append(xT)

            for b in range(BPC):
                xT = xTs[b]
                # ---- projections: qT/kT [h, t] (f32r for the QK matmul) ----
                qT_ps = psmall.tile([HS, T], f32, tag="ps")
                nc.tensor.matmul(qT_ps[:], wq_sb[:], xT[:])
                qT = bpool.tile([HS, T], f32r, tag="qT")
                nc.scalar.copy(qT[:], qT_ps[:])

                kT_ps = psmall.tile([HS, T], f32, tag="ps")
                nc.tensor.matmul(kT_ps[:], wk_sb[:], xT[:])
                kT = bpool.tile([HS, T], f32r, tag="kT")
                nc.scalar.copy(kT[:], kT_ps[:])

                # ---- v natural [u, d] + ones column, bf16 ----
                vp = bpool.tile([128, UB, HS + 1], bf16, tag="vp")
                for ub in range(UB):
                    v_ps = psmall.tile([128, HS], f32, tag="ps")
                    nc.tensor.matmul(
                        v_ps[:], xT[:, ub * 128 : (ub + 1) * 128], wv_sb[:]
                    )
                    nc.vector.tensor_copy(vp[:, ub, 0:HS], v_ps[:])
                nc.vector.memset(vp[:, :, HS : HS + 1], 1.0)

                # ---- QK transposed: weiT [u, t] = k @ q^T ----
                weiT = bpool.tile([128, UB, T], bf16, tag="weiT")
                for ub in range(UB):
                    weiT_ps = psmall.tile([128, T], f32, tag="ps")
                    nc.tensor.matmul(
                        weiT_ps[:], kT[:, ub * 128 : (ub + 1) * 128], qT[:]
                    )
                    if ub < 2:
                        nc.scalar.copy(weiT[:, ub], weiT_ps[:])
                    else:
                        nc.vector.tensor_copy(weiT[:, ub], weiT_ps[:])

                for qi in range(S // SQ):
                    # 2 MB bf16 load of SQ adjacency slices (1 KB lines)
                    adj4 = adjp.tile([128, SQ, UB, T], bf16, tag="adj")
                    src = adjT_d[b, SQ * qi : SQ * qi + SQ].rearrange(
                        "s (ub p) t -> p s ub t", p=128
                    )
                    nc.sync.dma_start(adj4[:], src)

                    # prodT = adjT * weiT (bf16 2x mode, weiT bcast over s)
                    prod = spool.tile([128, SQ, UB, T], bf16, tag="prod")
                    w_b = weiT[:].unsqueeze(1).broadcast_to((128, SQ, UB, T))
                    nc.vector.tensor_tensor(
                        prod[:], adj4[:], w_b, mybir.AluOpType.mult
                    )

                    # P^T = exp(scale * prodT)
                    pt = spool.tile([128, SQ, UB, T], bf16, tag="pt")
                    nc.scalar.activation(
                        pt[:], prod[:], mybir.ActivationFunctionType.Exp,
                        scale=SCALE,
                    )

                    outq = opool.tile([128, TB, SQ, HS], bf16, tag="outq")
                    for s2 in range(SQ):
                        # av [128, 1024] spans 2 PSUM banks; block (h, tb2) at
                        # h*512 + tb2*129 so no matmul output crosses a bank.
                        av = pav.tile([128, 2 * 512], f32, tag="av")
                        for h in range(2):
                            for tb2 in range(2):
                                tb = 2 * h + tb2
                                off = h * 512 + tb2 * 129
                                for ub in range(UB):
                                    nc.tensor.matmul(
                                        av[:, off : off + HS + 1],
                                        pt[:, s2, ub, tb * 128 : (tb + 1) * 128],
                                        vp[:, ub, :],
                                        start=(ub == 0),
                                        stop=(ub == UB - 1),
                                    )
                        # one strided reciprocal + one broadcast TT per slice
                        av_ap = av[:]
                        pdim = list(av_ap.ap[0])
                        sums = AP(
                            av_ap.tensor,
                            av_ap.offset + HS,
                            [pdim, [512, 2], [129, 2], [1, 1]],
                        )
                        rcp = tiny.tile([128, 2, 2], f32, tag="rcp")
                        nc.vector.reciprocal(rcp[:], sums)
                        vals = AP(
                            av_ap.tensor,
                            av_ap.offset,
                            [pdim, [512, 2], [129, 2], [1, HS]],
                        )
                        r_b = rcp[:].unsqueeze(3).broadcast_to((128, 2, 2, HS))
                        o_ap = outq[:, :, s2, :].rearrange(
                            "p (h t2) d -> p h t2 d", h=2
                        )
                        nc.vector.tensor_tensor(
                            o_ap, vals, r_b, mybir.AluOpType.mult
                        )

                    nc.gpsimd.dma_start(
                        out_d[b, :, SQ * qi : SQ * qi + SQ, :].rearrange(
                            "(tb p) s d -> p tb s d", p=128
                        ),
                        outq[:],
                    )

    nc.compile()
    return nc


def _get_module():
    global _CACHED
    if _CACHED is None:
        _CACHED = _build_module()
    return _CACHED


def run_on_hw(in_maps, trace=False, trace_kwargs=None):
    """Run the compiled module on the 8 NeuronCores. Returns BassKernelResults."""
    from concourse.bass_utils import run_bass_kernel_spmd
    from concourse.bass_interp import get_hw_module

    nc = _get_module()
    old_m = nc.m
    nc.m = get_hw_module(nc.m)
    try:
        return run_bass_kernel_spmd(
            nc,
            in_maps,
            core_ids=list(range(NCORES)),
            trace=trace,
            **(trace_kwargs or {}),
        )
    finally:
        nc.m = old_m


def make_in_maps(x, adjacent, Wq, Wk, Wv):
    x = np.ascontiguousarray(x, dtype=np.float32)
    Wq = np.ascontiguousarray(Wq, dtype=np.float32)
    Wk = np.ascontiguousarray(Wk, dtype=np.float32)
    Wv = np.ascontiguousarray(Wv, dtype=np.float32)
    xT = np.ascontiguousarray(x.transpose(0, 2, 1))  # [B, C, T]
    adjT = np.ascontiguousarray(
        np.asarray(adjacent, dtype=np.float32).transpose(0, 1, 3, 2)
    ).astype(ml_dtypes.bfloat16)  # [B, S, u, t] bf16
    return [
        {
            "xT": xT[c * BPC : (c + 1) * BPC],
            "adjT": adjT[c * BPC : (c + 1) * BPC],
            "Wq": Wq,
            "Wk": Wk,
            "Wv": Wv,
        }
        for c in range(NCORES)
    ]


def kernel(**inputs) -> np.ndarray:
    in_maps = make_in_maps(
        inputs["x"], inputs["adjacent"], inputs["Wq"], inputs["Wk"], inputs["Wv"]
    )
    res = run_on_hw(in_maps)
    # per-core out: [BPC, T, S, HS] bf16 -> [BPC, S, T, HS] f32
    outs = [
        np.asarray(res.results[c]["out"])
        .astype(np.float32)
        .transpose(0, 2, 1, 3)
        for c in range(NCORES)
    ]
    return np.ascontiguousarray(np.concatenate(outs, axis=0))


# revision 9
# speedup vs baseline: 1.1273x; 1.1273x over previous
"""Trainium2 Bass kernel for nn_Head_84043920048318 (sparse_attention).

Reference computation (per batch b):
    q = x @ Wq; k = x @ Wk; v = x @ Wv           [T, HS]
    wei = (q @ k.T) * C**-0.5                    [T, T]
    for s:  P = softmax(wei * adjacent[b, s], axis=-1);  out[b, s] = P @ v

Sharding: data-parallel over B across 8 NeuronCores (4 batches each);
projection weights replicated.

v4 design (everything lives in the transposed domain, no PE transposes):
  - host pre-transposes adjacent to [b, s, u, t] bf16 and x to [b, C, T];
    output leaves the device as [b, t, s, d] bf16 and is transposed/cast
    back on the host. All DMA lines are >= 1KB; adjacency DMA is 2 MB/op.
  - weiT [u, t] comes straight off the QK matmul with swapped operands
    (f32r), so the adjacency mask multiplies in its natural layout: one
    bf16 2x-mode DVE multiply + one ACT exp per FOUR s-slices.
  - AV matmuls use [v | 1] so the softmax denominator falls out of
    column 128 of PSUM; av blocks live at h*512 + tb2*129 in a 2-bank
    PSUM tile so one strided reciprocal + one broadcast tensor_tensor
    normalizes a whole s-slice (PSUM fp32 -> SBUF bf16).
  - input adjacency streams on the HWDGE (sync) queue; outputs, x and
    weights go via gpsimd SWDGE so they never block adjacency prefetch.
"""

import numpy as np
import ml_dtypes

B, S, T, C, HS = 32, 8, 512, 128, 128
NCORES = 8
BPC = B // NCORES
TB = T // 128
UB = T // 128
SCALE = float(C) ** -0.5
SQ = 4  # s-slices per adjacency DMA / mult / exp block

_CACHED = None


def _build_module():
    import concourse.bacc as bacc
    import concourse.mybir as mybir
    from concourse import tile
    from concourse.ap import AP

    f32 = mybir.dt.float32
    f32r = mybir.dt.float32r
    bf16 = mybir.dt.bfloat16

    nc = bacc.Bacc("TRN2", target_bir_lowering=False, debug=False, num_devices=1)

    xT_d = nc.dram_tensor("xT", [BPC, C, T], f32, kind="ExternalInput").ap()
    adjT_d = nc.dram_tensor("adjT", [BPC, S, T, T], bf16, kind="ExternalInput").ap()
    wq_d = nc.dram_tensor("Wq", [C, HS], f32, kind="ExternalInput").ap()
    wk_d = nc.dram_tensor("Wk", [C, HS], f32, kind="ExternalInput").ap()
    wv_d = nc.dram_tensor("Wv", [C, HS], f32, kind="ExternalInput").ap()
    # [b, t, s, d] so each DMA line is >= 4*HS contiguous = 1 KB bf16
    out_d = nc.dram_tensor("out", [BPC, T, S, HS], bf16, kind="ExternalOutput").ap()

    with tile.TileContext(nc) as tc:
        with (
            tc.tile_pool(name="consts", bufs=1) as consts,
            tc.tile_pool(name="xp", bufs=BPC) as xp,
            tc.tile_pool(name="bpool", bufs=2) as bpool,
            tc.tile_pool(name="opool", bufs=2) as opool,
            tc.tile_pool(name="adjp", bufs=3) as adjp,
            tc.tile_pool(name="spool", bufs=2) as spool,
            tc.tile_pool(name="tiny", bufs=8) as tiny,
            tc.tile_pool(name="pav", bufs=3, space="PSUM") as pav,
            tc.tile_pool(name="psmall", bufs=2, space="PSUM") as psmall,
        ):
            wq_sb = consts.tile([C, HS], f32, tag="wq")
            wk_sb = consts.tile([C, HS], f32, tag="wk")
            wv_sb = consts.tile([C, HS], f32, tag="wv")
            nc.gpsimd.dma_start(wq_sb[:], wq_d)
            nc.gpsimd.dma_start(wk_sb[:], wk_d)
            nc.gpsimd.dma_start(wv_sb[:], wv_d)

            # ---- x^T arrives pre-transposed [c, t]; all batches up front ----
            xTs = []
            for b in range(BPC):
                xT = xp.tile([C, T], f32, tag="xT")
                nc.gpsimd.dma_start(xT[:], xT_d[b])
                xTs.append(xT)

            for b in range(BPC):
                xT = xTs[b]
                # ---- projections: qT/kT [h, t] (f32r for the QK matmul) ----
                qT_ps = psmall.tile([HS, T], f32, tag="ps")
                nc.tensor.matmul(qT_ps[:], wq_sb[:], xT[:])
                qT = bpool.tile([HS, T], f32r, tag="qT")
                nc.scalar.copy(qT[:], qT_ps[:])

                kT_ps = psmall.tile([HS, T], f32, tag="ps")
                nc.tensor.matmul(kT_ps[:], wk_sb[:], xT[:])
                kT = bpool.tile([HS, T], f32r, tag="kT")
                nc.scalar.copy(kT[:], kT_ps[:])

                # ---- v natural [u, d] + ones column, bf16 ----
                vp = bpool.tile([128, UB, HS + 1], bf16, tag="vp")
                for ub in range(UB):
                    v_ps = psmall.tile([128, HS], f32, tag="ps")
                    nc.tensor.matmul(
                        v_ps[:], xT[:, ub * 128 : (ub + 1) * 128], wv_sb[:]
                    )
                    nc.vector.tensor_copy(vp[:, ub, 0:HS], v_ps[:])
                nc.vector.memset(vp[:, :, HS : HS + 1], 1.0)

                # ---- QK transposed: weiT [u, t] = k @ q^T ----
                weiT = bpool.tile([128, UB, T], bf16, tag="weiT")
                for ub in range(UB):
                    weiT_ps = psmall.tile([128, T], f32, tag="ps")
                    nc.tensor.matmul(
                        weiT_ps[:], kT[:, ub * 128 : (ub + 1) * 128], qT[:]
                    )
                    if ub < 2:
                        nc.scalar.copy(weiT[:, ub], weiT_ps[:])
                    else:
                        nc.vector.tensor_copy(weiT[:, ub], weiT_ps[:])

                for qi in range(S // SQ):
                    # 2 MB bf16 load of SQ adjacency slices (1 KB lines)
                    adj4 = adjp.tile([128, SQ, UB, T], bf16, tag="adj")
                    src = adjT_d[b, SQ * qi : SQ * qi + SQ].rearrange(
                        "s (ub p) t -> p s ub t", p=128
                    )
                    nc.sync.dma_start(adj4[:], src)

                    outq = opool.tile([128, TB, SQ, HS], bf16, tag="outq")
                    for half in range(SQ // 2):
                        # prodT = adjT * weiT (bf16 2x, weiT bcast over s)
                        prod = spool.tile([128, 2, UB, T], bf16, tag="prod")
                        w_b = weiT[:].unsqueeze(1).broadcast_to((128, 2, UB, T))
                        nc.vector.tensor_tensor(
                            prod[:],
                            adj4[:, 2 * half : 2 * half + 2],
                            w_b,
                            mybir.AluOpType.mult,
                        )

                        # P^T = exp(scale * prodT)
                        pt = spool.tile([128, 2, UB, T], bf16, tag="pt")
                        nc.scalar.activation(
                            pt[:], prod[:], mybir.ActivationFunctionType.Exp,
                            scale=SCALE,
                        )

                        for s2 in range(2):
                            sq = 2 * half + s2
                            # av [128, 1024] spans 2 PSUM banks; block
                            # (h, tb2) at h*512 + tb2*129 so no matmul
                            # output crosses a bank.
                            av = pav.tile([128, 2 * 512], f32, tag="av")
                            for h in range(2):
                                for tb2 in range(2):
                                    tb = 2 * h + tb2
                                    off = h * 512 + tb2 * 129
                                    for ub in range(UB):
                                        nc.tensor.matmul(
                                            av[:, off : off + HS + 1],
                                            pt[
                                                :,
                                                s2,
                                                ub,
                                                tb * 128 : (tb + 1) * 128,
                                            ],
                                            vp[:, ub, :],
                                            start=(ub == 0),
                                            stop=(ub == UB - 1),
                                        )
                            # one strided reciprocal + one broadcast TT
                            av_ap = av[:]
                            pdim = list(av_ap.ap[0])
                            sums = AP(
                                av_ap.tensor,
                                av_ap.offset + HS,
                                [pdim, [512, 2], [129, 2], [1, 1]],
                            )
                            rcp = tiny.tile([128, 2, 2], f32, tag="rcp")
                            nc.vector.reciprocal(rcp[:], sums)
                            vals = AP(
                                av_ap.tensor,
                                av_ap.offset,
                                [pdim, [512, 2], [129, 2], [1, HS]],
                            )
                            r_b = rcp[:].unsqueeze(3).broadcast_to(
                                (128, 2, 2, HS)
                            )
                            o_ap = outq[:, :, sq, :].rearrange(
                                "p (h t2) d -> p h t2 d", h=2
                            )
                            nc.vector.tensor_tensor(
                                o_ap, vals, r_b, mybir.AluOpType.mult
                            )

                    nc.gpsimd.dma_start(
                        out_d[b, :, SQ * qi : SQ * qi + SQ, :].rearrange(
                            "(tb p) s d -> p tb s d", p=128
                        ),
                        outq[:],
                    )

    nc.compile()
    return nc


def _get_module():
    global _CACHED
    if _CACHED is None:
        _CACHED = _build_module()
    return _CACHED


def run_on_hw(in_maps, trace=False, trace_kwargs=None):
    """Run the compiled module on the 8 NeuronCores. Returns BassKernelResults."""
    from concourse.bass_utils import run_bass_kernel_spmd
    from concourse.bass_interp import get_hw_module

    nc = _get_module()
    old_m = nc.m
    nc.m = get_hw_module(nc.m)
    try:
        return run_bass_kernel_spmd(
            nc,
            in_maps,
            core_ids=list(range(NCORES)),
            trace=trace,
            **(trace_kwargs or {}),
        )
    finally:
        nc.m = old_m


def make_in_maps(x, adjacent, Wq, Wk, Wv):
    x = np.ascontiguousarray(x, dtype=np.float32)
    Wq = np.ascontiguousarray(Wq, dtype=np.float32)
    Wk = np.ascontiguousarray(Wk, dtype=np.float32)
    Wv = np.ascontiguousarray(Wv, dtype=np.float32)
    xT = np.ascontiguousarray(x.transpose(0, 2, 1))  # [B, C, T]
    adjT = np.ascontiguousarray(
        np.asarray(adjacent, dtype=np.float32).transpose(0, 1, 3, 2)
    ).astype(ml_dtypes.bfloat16)  # [B, S, u, t] bf16
    return [
        {
            "xT": xT[c * BPC : (c + 1) * BPC],
            "adjT": adjT[c * BPC : (c + 1) * BPC],
            "Wq": Wq,
            "Wk": Wk,
            "Wv": Wv,
        }
        for c in range(NCORES)
    ]


def kernel(**inputs) -> np.ndarray:
    in_maps = make_in_maps(
        inputs["x"], inputs["adjacent"], inputs["Wq"], inputs["Wk"], inputs["Wv"]
    )
    res = run_on_hw(in_maps)
    # per-core out: [BPC, T, S, HS] bf16 -> [BPC, S, T, HS] f32
    outs = [
        np.asarray(res.results[c]["out"])
        .astype(np.float32)
        .transpose(0, 2, 1, 3)
        for c in range(NCORES)
    ]
    return np.ascontiguousarray(np.concatenate(outs, axis=0))
